# revision 6
# baseline (speedup 1.0000x reference)
"""Trainium2 Bass kernel for the ConvolutionalKAN problem.

Math: the KAN conv
    out[b,o,y,x] = sum_{j,kk,l,m} phi_m(11*inp[b,j,y+kk,x+l]) * coeff[o,j,kk,l,m]
with phi_m the degree-3 B-spline basis on uniform knots linspace(0,1,12).
Uniform knots -> phi_m(t) = N3(t-m), branch-free (u = |t-(m+2)|):
    6*N3 = relu(2-u)^3 - 4*relu(1-u)^3
This kernel evaluates the NEGATED basis with clamp/square tricks so every
vector op is a cheap bf16 DVE pass (2x/4x perf modes):
    u    = |11x + b|          (Scalar, per-partition bias, bf16 out)
    sq4  = (2-2u)^2           (Scalar Square, = 4(1-u)^2)
    an   = min(u-2, 0)        (DVE tensor_scalar imm, 4x mode; = -relu(2-u))
    m1   = min(u-1, 0)        (DVE 4x; = -relu(1-u))
    sa   = an*an              (GpSimd mult; = relu(2-u)^2)
    a3   = sa*an              (DVE 2x; = -relu(2-u)^3)
    r3   = sq4*m1             (DVE 2x; = -4*relu(1-u)^3)
    g    = a3 - r3            (DVE 2x; = -(6*N3))
The sign is folded into the host-prepared weights (w = -coeff/6), so the
matmul chains (4 contraction tiles x 9 taps, bf16, N = rows*62 per PSUM
bank) are unchanged in count but all-bf16: TensorE streams 1 col/cycle at
2.4 GHz and becomes the bottleneck (~117us), with Vector ~90us, Scalar
~90us, GpSimd ~80us hidden behind it.

Sharding: data-parallel over batch, 2 images per core on 8 cores.
"""

import os
import sys

import numpy as np

for _p in ("/root/.axon_site/_ro/trn_rl_repo", "/opt/trn_rl_repo"):
    if os.path.isdir(_p) and _p not in sys.path:
        sys.path.append(_p)

B_FULL = 16
N_CORES = 8
B_SHARD = B_FULL // N_CORES
CIN = 64
COUT = 64
H = 64
W = 64
KS = 3
NB = 8
HO = H - KS + 1  # 62
WO = W - KS + 1  # 62
NQ = (CIN * NB) // 128  # 4 contraction tiles of 128
TAPS = KS * KS

# output row strips: (y0, n_input_rows, per-group output rows)
STRIPS = [(0, 10, (8,)), (8, 10, (8,)), (16, 18, (8, 8)), (32, 18, (8, 8)),
          (48, 16, (8, 6))]


def _fold_coeff(coeff: np.ndarray):
    """coeff [COUT, CIN, KS, KS, NB] -> W_host [512, 576] f32 (negated).

    The on-chip basis is g = -(6*phi), so weights fold to -coeff/6 in
    layout W_host[m*64 + j, (kk*3+l)*64 + o].
    """
    w = (-coeff.astype(np.float64).transpose(4, 1, 2, 3, 0) / 6.0).reshape(
        NB * CIN, TAPS * COUT)
    return np.ascontiguousarray(w, dtype=np.float32)


def _build_bass():
    import concourse.bacc as bacc
    import concourse.mybir as mybir
    import concourse.tile as tile

    f32 = mybir.dt.float32
    bf16 = mybir.dt.bfloat16
    AF = mybir.ActivationFunctionType
    alu = mybir.AluOpType

    nc = bacc.Bacc("TRN2", target_bir_lowering=False, debug=False,
                   num_devices=N_CORES)
    x_d = nc.dram_tensor("x", [B_SHARD, CIN, H, W], f32, kind="ExternalInput").ap()
    w_d = nc.dram_tensor("w", [NB * CIN, TAPS * COUT], f32, kind="ExternalInput").ap()
    b_d = nc.dram_tensor("btbl", [128, NQ + 1], f32, kind="ExternalInput").ap()
    out_d = nc.dram_tensor("out", [B_SHARD, COUT, HO, WO], f32,
                           kind="ExternalOutput").ap()

    with tile.TileContext(nc) as tc:
        from contextlib import ExitStack

        with ExitStack() as ctx:
            wpool = ctx.enter_context(tc.tile_pool(name="w", bufs=NQ))
            cpool = ctx.enter_context(tc.tile_pool(name="const", bufs=1))
            xpool = ctx.enter_context(tc.tile_pool(name="x", bufs=3))
            gpool = ctx.enter_context(tc.tile_pool(name="g", bufs=3 * NQ))
            ipool = ctx.enter_context(tc.tile_pool(name="i", bufs=3))
            opool = ctx.enter_context(tc.tile_pool(name="o", bufs=4))
            ppool = ctx.enter_context(
                tc.tile_pool(name="ps", bufs=4, space="PSUM"))

            bt = cpool.tile([128, NQ + 1], f32)
            nc.sync.dma_start(bt[:], b_d[:])
            wts = []
            for q in range(NQ):
                wt = wpool.tile([128, TAPS * COUT], f32, tag="wstage")
                nc.sync.dma_start(wt[:], w_d[q * 128:(q + 1) * 128, :])
                wr = wpool.tile([128, TAPS * COUT], bf16, tag="wr")
                nc.vector.tensor_copy(wr[:], wt[:])
                wts.append(wr)

            NPIX = 18 * W
            for b in range(B_SHARD):
                for (y0, nin, groups) in STRIPS:
                    npx = nin * W
                    xt = xpool.tile([128, NPIX], f32)
                    src = x_d[b, :, y0:y0 + nin, :]
                    nc.gpsimd.dma_start(
                        xt[0:64].rearrange("p (r c) -> p r c", c=W)[:, :nin, :],
                        src)
                    nc.gpsimd.dma_start(
                        xt[64:128].rearrange("p (r c) -> p r c", c=W)[:, :nin, :],
                        src)
                    gts = []
                    for q in range(NQ):
                        u = ipool.tile([128, NPIX], bf16, tag="u")
                        sq4 = ipool.tile([128, NPIX], bf16, tag="sq4")
                        an = ipool.tile([128, NPIX], bf16, tag="an")
                        m1 = ipool.tile([128, NPIX], bf16, tag="m1")
                        sa = ipool.tile([128, NPIX], bf16, tag="sa")
                        a3 = ipool.tile([128, NPIX], bf16, tag="a3")
                        r3 = ipool.tile([128, NPIX], bf16, tag="r3")
                        g = gpool.tile([128, NPIX], bf16)
                        # u = |11x - (m+2)|, m = 2q + (p>=64)
                        nc.scalar.activation(u[:, :npx], xt[:, :npx],
                                             AF.Abs, bias=bt[:, q:q + 1],
                                             scale=11.0)
                        # sq4 = (2-2u)^2 = 4(1-u)^2
                        nc.scalar.activation(sq4[:, :npx], u[:, :npx],
                                             AF.Square, bias=bt[:, NQ:NQ + 1],
                                             scale=-2.0)
                        # an = min(u-2,0) = -relu(2-u)   [DVE 4x]
                        nc.vector.tensor_scalar(an[:, :npx], u[:, :npx],
                                                2.0, 0.0, op0=alu.subtract,
                                                op1=alu.min)
                        # m1 = min(u-1,0) = -relu(1-u)
                        if q % 2 == 0:
                            nc.gpsimd.tensor_scalar(m1[:, :npx], u[:, :npx],
                                                    1.0, 0.0, op0=alu.subtract,
                                                    op1=alu.min)
                        else:
                            nc.vector.tensor_scalar(m1[:, :npx], u[:, :npx],
                                                    1.0, 0.0, op0=alu.subtract,
                                                    op1=alu.min)
                        # sa = relu(2-u)^2: GpSimd computes an*an for odd q;
                        # Scalar computes Square(2-u) for even q (equal to
                        # an*an wherever an != 0, and multiplied by an=0 else)
                        if q % 2 == 0:
                            nc.scalar.activation(sa[:, :npx], u[:, :npx],
                                                 AF.Square, bias=bt[:, NQ:NQ + 1],
                                                 scale=-1.0)
                        else:
                            nc.gpsimd.tensor_tensor(sa[:, :npx], an[:, :npx],
                                                    an[:, :npx], op=alu.mult)
                        # a3 = sa*an = -relu(2-u)^3      [DVE 2x]
                        nc.vector.tensor_tensor(a3[:, :npx], sa[:, :npx],
                                                an[:, :npx], op=alu.mult)
                        # r3 = sq4*m1 = -4relu(1-u)^3    [DVE 2x]
                        nc.vector.tensor_tensor(r3[:, :npx], sq4[:, :npx],
                                                m1[:, :npx], op=alu.mult)
                        # g = a3 - r3 = -(6*N3)          [DVE 2x]
                        nc.vector.tensor_tensor(g[:, :npx], a3[:, :npx],
                                                r3[:, :npx], op=alu.subtract)
                        gts.append(g)

                    gvs = [g[:].rearrange("p (r c) -> p r c", c=W) for g in gts]
                    n_mm = NQ * TAPS
                    for grp, nr in enumerate(groups):
                        ps = ppool.tile([64, 8, WO], f32)
                        i_mm = 0
                        for q in range(NQ):
                            for kk in range(KS):
                                for l in range(KS):
                                    r0 = 8 * grp + kk
                                    rhs = gvs[q][:, r0:r0 + nr, l:l + WO]
                                    lhsT = wts[q][:, (kk * KS + l) * COUT:
                                                  (kk * KS + l + 1) * COUT]
                                    nc.tensor.matmul(
                                        ps[:, :nr, :], lhsT, rhs,
                                        start=(i_mm == 0),
                                        stop=(i_mm == n_mm - 1),
                                    )
                                    i_mm += 1
                        ot = opool.tile([64, 8, WO], f32)
                        nc.scalar.activation(ot[:, :nr, :], ps[:, :nr, :],
                                             AF.Identity, scale=1.0)
                        nc.sync.dma_start(
                            out_d[b, :, y0 + 8 * grp:y0 + 8 * grp + nr, :],
                            ot[:, :nr, :])

    nc.compile()
    return nc


def _maybe_install_profile_shim():
    """Allow trace=True/BASS_TRACE under axon even though this image lacks
    antenv.axon_hooks; degrade silently if anything is missing."""
    import types

    if "antenv.axon_hooks" in sys.modules:
        return
    try:
        from trn_agent_boot.trn_boot import _ntff_profile_via_ctypes

        hook = _ntff_profile_via_ctypes("/opt/axon/libaxon_pjrt.so")
        if hook is None:
            return
        mod = types.ModuleType("antenv.axon_hooks")
        mod.get_axon_ntff_profile_hook = lambda: hook
        mod.set_axon_ntff_profile_hook = lambda h: None
        sys.modules["antenv.axon_hooks"] = mod
        from concourse import bass_utils

        bass_utils.upload_artifacts = lambda tmpdir: f"local:{tmpdir}"
    except Exception:
        pass


_LAST_RESULTS = None


def kernel(x: np.ndarray, coeff: np.ndarray) -> np.ndarray:
    global _LAST_RESULTS
    from concourse import bass_utils

    _maybe_install_profile_shim()

    x = np.ascontiguousarray(np.asarray(x), dtype=np.float32)
    coeff = np.asarray(coeff)
    assert x.shape == (B_FULL, CIN, H, W), x.shape

    w_host = _fold_coeff(coeff)
    btbl = np.zeros((128, NQ + 1), dtype=np.float32)
    for p in range(128):
        for q in range(NQ):
            m = 2 * q + (1 if p >= 64 else 0)
            btbl[p, q] = -float(m + 2)
    btbl[:, NQ] = 2.0

    nc = _build_bass()

    in_maps = []
    for i in range(N_CORES):
        in_maps.append({
            "x": np.ascontiguousarray(x[i * B_SHARD:(i + 1) * B_SHARD]),
            "w": w_host,
            "btbl": btbl,
        })

    res = bass_utils.run_bass_kernel_spmd(
        nc, in_maps, core_ids=list(range(N_CORES)),
        trace=bool(os.environ.get("KAN_TRACE")),
    )
    _LAST_RESULTS = res

    out = np.concatenate([res.results[i]["out"] for i in range(N_CORES)], axis=0)
    return out.astype(np.float32, copy=False)


# revision 7
# speedup vs baseline: 2.4352x; 2.4352x over previous
"""Trainium2 Bass kernel for the ConvolutionalKAN problem.

Math: the KAN conv
    out[b,o,y,x] = sum_{j,kk,l,m} phi_m(11*inp[b,j,y+kk,x+l]) * coeff[o,j,kk,l,m]
with phi_m the degree-3 B-spline basis on uniform knots linspace(0,1,12).
Uniform knots -> phi_m(t) = N3(t-m), branch-free (u = |t-(m+2)|):
    6*N3 = relu(2-u)^3 - 4*relu(1-u)^3
This kernel evaluates the NEGATED basis with clamp/square tricks so every
vector op is a cheap bf16 DVE pass (2x/4x perf modes):
    u    = |11x + b|          (Scalar, per-partition bias, bf16 out)
    sq4  = (2-2u)^2           (Scalar Square, = 4(1-u)^2)
    an   = min(u-2, 0)        (DVE tensor_scalar imm, 4x mode; = -relu(2-u))
    m1   = min(u-1, 0)        (DVE 4x; = -relu(1-u))
    sa   = an*an              (GpSimd mult; = relu(2-u)^2)
    a3   = sa*an              (DVE 2x; = -relu(2-u)^3)
    r3   = sq4*m1             (DVE 2x; = -4*relu(1-u)^3)
    g    = a3 - r3            (DVE 2x; = -(6*N3))
The sign is folded into the host-prepared weights (w = -coeff/6), so the
matmul chains (4 contraction tiles x 9 taps, bf16, N = rows*62 per PSUM
bank) are unchanged in count but all-bf16: TensorE streams 1 col/cycle at
2.4 GHz and becomes the bottleneck (~117us), with Vector ~90us, Scalar
~90us, GpSimd ~80us hidden behind it.

Sharding: data-parallel over batch, 2 images per core on 8 cores.
"""

import os
import sys

import numpy as np

for _p in ("/root/.axon_site/_ro/trn_rl_repo", "/opt/trn_rl_repo"):
    if os.path.isdir(_p) and _p not in sys.path:
        sys.path.append(_p)

B_FULL = 16
N_CORES = 8
B_SHARD = B_FULL // N_CORES
CIN = 64
COUT = 64
H = 64
W = 64
KS = 3
NB = 8
HO = H - KS + 1  # 62
WO = W - KS + 1  # 62
NQ = (CIN * NB) // 128  # 4 contraction tiles of 128
TAPS = KS * KS

# output row strips: (y0, n_input_rows, per-group output rows)
STRIPS = [(0, 10, (8,)), (8, 10, (8,)), (16, 18, (8, 8)), (32, 18, (8, 8)),
          (48, 16, (8, 6))]


def _fold_coeff(coeff: np.ndarray):
    """coeff [COUT, CIN, KS, KS, NB] -> W_host [512, 576] f32 (negated).

    The on-chip basis is g = -(6*phi), so weights fold to -coeff/6 in
    layout W_host[m*64 + j, (kk*3+l)*64 + o].
    """
    w = (-coeff.astype(np.float64).transpose(4, 1, 2, 3, 0) / 6.0).reshape(
        NB * CIN, TAPS * COUT)
    return np.ascontiguousarray(w, dtype=np.float32)


def _build_bass():
    import concourse.bacc as bacc
    import concourse.mybir as mybir
    import concourse.tile as tile

    f32 = mybir.dt.float32
    bf16 = mybir.dt.bfloat16
    AF = mybir.ActivationFunctionType
    alu = mybir.AluOpType

    nc = bacc.Bacc("TRN2", target_bir_lowering=False, debug=False,
                   num_devices=N_CORES)
    x_d = nc.dram_tensor("x", [B_SHARD, CIN, H, W], f32, kind="ExternalInput").ap()
    w_d = nc.dram_tensor("w", [NB * CIN, TAPS * COUT], f32, kind="ExternalInput").ap()
    b_d = nc.dram_tensor("btbl", [128, NQ + 1], f32, kind="ExternalInput").ap()
    out_d = nc.dram_tensor("out", [B_SHARD, COUT, HO, WO], f32,
                           kind="ExternalOutput").ap()

    with tile.TileContext(nc) as tc:
        from contextlib import ExitStack

        with ExitStack() as ctx:
            wpool = ctx.enter_context(tc.tile_pool(name="w", bufs=NQ))
            cpool = ctx.enter_context(tc.tile_pool(name="const", bufs=1))
            xpool = ctx.enter_context(tc.tile_pool(name="x", bufs=3))
            gpool = ctx.enter_context(tc.tile_pool(name="g", bufs=3 * NQ))
            ipool = ctx.enter_context(tc.tile_pool(name="i", bufs=3))
            opool = ctx.enter_context(tc.tile_pool(name="o", bufs=4))
            ppool = ctx.enter_context(
                tc.tile_pool(name="ps", bufs=4, space="PSUM"))

            bt = cpool.tile([128, NQ + 1], f32)
            nc.sync.dma_start(bt[:], b_d[:])
            wts = []
            for q in range(NQ):
                wt = wpool.tile([128, TAPS * COUT], f32, tag="wstage")
                nc.sync.dma_start(wt[:], w_d[q * 128:(q + 1) * 128, :])
                wr = wpool.tile([128, TAPS * COUT], bf16, tag="wr")
                nc.vector.tensor_copy(wr[:], wt[:])
                wts.append(wr)

            NPIX = 18 * W
            for b in range(B_SHARD):
                for (y0, nin, groups) in STRIPS:
                    npx = nin * W
                    xt = xpool.tile([128, NPIX], f32)
                    src = x_d[b, :, y0:y0 + nin, :]
                    nc.gpsimd.dma_start(
                        xt[0:64].rearrange("p (r c) -> p r c", c=W)[:, :nin, :],
                        src)
                    nc.gpsimd.dma_start(
                        xt[64:128].rearrange("p (r c) -> p r c", c=W)[:, :nin, :],
                        src)
                    gts = []
                    for q in range(NQ):
                        u = ipool.tile([128, NPIX], bf16, tag="u")
                        sq4 = ipool.tile([128, NPIX], bf16, tag="sq4")
                        an = ipool.tile([128, NPIX], bf16, tag="an")
                        m1 = ipool.tile([128, NPIX], bf16, tag="m1")
                        sa = ipool.tile([128, NPIX], bf16, tag="sa")
                        a3 = ipool.tile([128, NPIX], bf16, tag="a3")
                        r3 = ipool.tile([128, NPIX], bf16, tag="r3")
                        g = gpool.tile([128, NPIX], bf16)
                        # u = |11x - (m+2)|, m = 2q + (p>=64)
                        nc.scalar.activation(u[:, :npx], xt[:, :npx],
                                             AF.Abs, bias=bt[:, q:q + 1],
                                             scale=11.0)
                        # sq4 = (2-2u)^2 = 4(1-u)^2
                        nc.scalar.activation(sq4[:, :npx], u[:, :npx],
                                             AF.Square, bias=bt[:, NQ:NQ + 1],
                                             scale=-2.0)
                        # an = min(u-2,0) = -relu(2-u)   [DVE 4x]
                        nc.vector.tensor_scalar(an[:, :npx], u[:, :npx],
                                                2.0, 0.0, op0=alu.subtract,
                                                op1=alu.min)
                        # m1 = min(u-1,0) = -relu(1-u)   [DVE]
                        nc.vector.tensor_scalar(m1[:, :npx], u[:, :npx],
                                                1.0, 0.0, op0=alu.subtract,
                                                op1=alu.min)
                        # sa = relu(2-u)^2: GpSimd computes an*an for odd q;
                        # Scalar computes Square(2-u) for even q (equal to
                        # an*an wherever an != 0, and multiplied by an=0 else)
                        if q % 2 == 0:
                            nc.scalar.activation(sa[:, :npx], u[:, :npx],
                                                 AF.Square, bias=bt[:, NQ:NQ + 1],
                                                 scale=-1.0)
                        else:
                            nc.gpsimd.tensor_tensor(sa[:, :npx], an[:, :npx],
                                                    an[:, :npx], op=alu.mult)
                        # a3 = sa*an = -relu(2-u)^3      [DVE 2x]
                        nc.vector.tensor_tensor(a3[:, :npx], sa[:, :npx],
                                                an[:, :npx], op=alu.mult)
                        # r3 = sq4*m1 = -4relu(1-u)^3    [DVE 2x]
                        nc.vector.tensor_tensor(r3[:, :npx], sq4[:, :npx],
                                                m1[:, :npx], op=alu.mult)
                        # g = a3 - r3 = -(6*N3)          [DVE 2x]
                        nc.vector.tensor_tensor(g[:, :npx], a3[:, :npx],
                                                r3[:, :npx], op=alu.subtract)
                        gts.append(g)

                    gvs = [g[:].rearrange("p (r c) -> p r c", c=W) for g in gts]
                    n_mm = NQ * TAPS
                    for grp, nr in enumerate(groups):
                        ps = ppool.tile([64, 8, WO], f32)
                        i_mm = 0
                        for q in range(NQ):
                            for kk in range(KS):
                                for l in range(KS):
                                    r0 = 8 * grp + kk
                                    rhs = gvs[q][:, r0:r0 + nr, l:l + WO]
                                    lhsT = wts[q][:, (kk * KS + l) * COUT:
                                                  (kk * KS + l + 1) * COUT]
                                    nc.tensor.matmul(
                                        ps[:, :nr, :], lhsT, rhs,
                                        start=(i_mm == 0),
                                        stop=(i_mm == n_mm - 1),
                                    )
                                    i_mm += 1
                        ot = opool.tile([64, 8, WO], f32)
                        nc.scalar.activation(ot[:, :nr, :], ps[:, :nr, :],
                                             AF.Identity, scale=1.0)
                        nc.sync.dma_start(
                            out_d[b, :, y0 + 8 * grp:y0 + 8 * grp + nr, :],
                            ot[:, :nr, :])

    nc.compile()
    return nc


def _maybe_install_profile_shim():
    """Allow trace=True/BASS_TRACE under axon even though this image lacks
    antenv.axon_hooks; degrade silently if anything is missing."""
    import types

    if "antenv.axon_hooks" in sys.modules:
        return
    try:
        from trn_agent_boot.trn_boot import _ntff_profile_via_ctypes

        hook = _ntff_profile_via_ctypes("/opt/axon/libaxon_pjrt.so")
        if hook is None:
            return
        mod = types.ModuleType("antenv.axon_hooks")
        mod.get_axon_ntff_profile_hook = lambda: hook
        mod.set_axon_ntff_profile_hook = lambda h: None
        sys.modules["antenv.axon_hooks"] = mod
        from concourse import bass_utils

        bass_utils.upload_artifacts = lambda tmpdir: f"local:{tmpdir}"
    except Exception:
        pass


_LAST_RESULTS = None


def kernel(x: np.ndarray, coeff: np.ndarray) -> np.ndarray:
    global _LAST_RESULTS
    from concourse import bass_utils

    _maybe_install_profile_shim()

    x = np.ascontiguousarray(np.asarray(x), dtype=np.float32)
    coeff = np.asarray(coeff)
    assert x.shape == (B_FULL, CIN, H, W), x.shape

    w_host = _fold_coeff(coeff)
    btbl = np.zeros((128, NQ + 1), dtype=np.float32)
    for p in range(128):
        for q in range(NQ):
            m = 2 * q + (1 if p >= 64 else 0)
            btbl[p, q] = -float(m + 2)
    btbl[:, NQ] = 2.0

    nc = _build_bass()

    in_maps = []
    for i in range(N_CORES):
        in_maps.append({
            "x": np.ascontiguousarray(x[i * B_SHARD:(i + 1) * B_SHARD]),
            "w": w_host,
            "btbl": btbl,
        })

    res = bass_utils.run_bass_kernel_spmd(
        nc, in_maps, core_ids=list(range(N_CORES)),
        trace=bool(os.environ.get("KAN_TRACE")),
    )
    _LAST_RESULTS = res

    out = np.concatenate([res.results[i]["out"] for i in range(N_CORES)], axis=0)
    return out.astype(np.float32, copy=False)


# revision 8
# speedup vs baseline: 2.5308x; 1.0392x over previous
"""Trainium2 Bass kernel for the ConvolutionalKAN problem.

Math: the KAN conv
    out[b,o,y,x] = sum_{j,kk,l,m} phi_m(11*inp[b,j,y+kk,x+l]) * coeff[o,j,kk,l,m]
with phi_m the degree-3 B-spline basis on uniform knots linspace(0,1,12).
Uniform knots -> phi_m(t) = N3(t-m), branch-free (u = |t-(m+2)|):
    6*N3 = relu(2-u)^3 - 4*relu(1-u)^3
This kernel evaluates the NEGATED basis with clamp/square tricks so every
vector op is a cheap bf16 DVE pass (2x/4x perf modes):
    u    = |11x + b|          (Scalar, per-partition bias, bf16 out)
    sq4  = (2-2u)^2           (Scalar Square, = 4(1-u)^2)
    an   = min(u-2, 0)        (DVE tensor_scalar imm, 4x mode; = -relu(2-u))
    m1   = min(u-1, 0)        (DVE 4x; = -relu(1-u))
    sa   = an*an              (GpSimd mult; = relu(2-u)^2)
    a3   = sa*an              (DVE 2x; = -relu(2-u)^3)
    r3   = sq4*m1             (DVE 2x; = -4*relu(1-u)^3)
    g    = a3 - r3            (DVE 2x; = -(6*N3))
The sign is folded into the host-prepared weights (w = -coeff/6), so the
matmul chains (4 contraction tiles x 9 taps, bf16, N = rows*62 per PSUM
bank) are unchanged in count but all-bf16: TensorE streams 1 col/cycle at
2.4 GHz and becomes the bottleneck (~117us), with Vector ~90us, Scalar
~90us, GpSimd ~80us hidden behind it.

Sharding: data-parallel over batch, 2 images per core on 8 cores.
"""

import os
import sys

import numpy as np

for _p in ("/root/.axon_site/_ro/trn_rl_repo", "/opt/trn_rl_repo"):
    if os.path.isdir(_p) and _p not in sys.path:
        sys.path.append(_p)

B_FULL = 16
N_CORES = 8
B_SHARD = B_FULL // N_CORES
CIN = 64
COUT = 64
H = 64
W = 64
KS = 3
NB = 8
HO = H - KS + 1  # 62
WO = W - KS + 1  # 62
NQ = (CIN * NB) // 128  # 4 contraction tiles of 128
TAPS = KS * KS

# output row strips: (y0, n_input_rows, per-group output rows)
STRIPS = [(0, 18, (8, 8)), (16, 18, (8, 8)), (32, 18, (8, 8)), (48, 16, (8, 6))]


def _fold_coeff(coeff: np.ndarray):
    """coeff [COUT, CIN, KS, KS, NB] -> W_host [512, 576] f32 (negated).

    The on-chip basis is g = -(6*phi), so weights fold to -coeff/6 in
    layout W_host[m*64 + j, (kk*3+l)*64 + o].
    """
    w = (-coeff.astype(np.float64).transpose(4, 1, 2, 3, 0) / 6.0).reshape(
        NB * CIN, TAPS * COUT)
    return np.ascontiguousarray(w, dtype=np.float32)


def _build_bass():
    import concourse.bacc as bacc
    import concourse.mybir as mybir
    import concourse.tile as tile

    f32 = mybir.dt.float32
    bf16 = mybir.dt.bfloat16
    AF = mybir.ActivationFunctionType
    alu = mybir.AluOpType

    nc = bacc.Bacc("TRN2", target_bir_lowering=False, debug=False,
                   num_devices=N_CORES)
    x_d = nc.dram_tensor("x", [B_SHARD, CIN, H, W], f32, kind="ExternalInput").ap()
    w_d = nc.dram_tensor("w", [NB * CIN, TAPS * COUT], f32, kind="ExternalInput").ap()
    b_d = nc.dram_tensor("btbl", [128, NQ + 1], f32, kind="ExternalInput").ap()
    out_d = nc.dram_tensor("out", [B_SHARD, COUT, HO, WO], f32,
                           kind="ExternalOutput").ap()

    with tile.TileContext(nc) as tc:
        from contextlib import ExitStack

        with ExitStack() as ctx:
            wpool = ctx.enter_context(tc.tile_pool(name="w", bufs=NQ))
            cpool = ctx.enter_context(tc.tile_pool(name="const", bufs=1))
            xpool = ctx.enter_context(tc.tile_pool(name="x", bufs=3))
            gpool = ctx.enter_context(tc.tile_pool(name="g", bufs=3 * NQ))
            ipool = ctx.enter_context(tc.tile_pool(name="i", bufs=3))
            opool = ctx.enter_context(tc.tile_pool(name="o", bufs=4))
            ppool = ctx.enter_context(
                tc.tile_pool(name="ps", bufs=4, space="PSUM"))

            bt = cpool.tile([128, NQ + 1], f32)
            nc.sync.dma_start(bt[:], b_d[:])
            wts = []
            for q in range(NQ):
                wt = wpool.tile([128, TAPS * COUT], f32, tag="wstage")
                nc.sync.dma_start(wt[:], w_d[q * 128:(q + 1) * 128, :])
                wr = wpool.tile([128, TAPS * COUT], bf16, tag="wr")
                nc.vector.tensor_copy(wr[:], wt[:])
                wts.append(wr)

            NPIX = 18 * W
            for b in range(B_SHARD):
                for (y0, nin, groups) in STRIPS:
                    npx = nin * W
                    xt = xpool.tile([128, NPIX], f32)
                    src = x_d[b, :, y0:y0 + nin, :]
                    nc.sync.dma_start(
                        xt[0:64].rearrange("p (r c) -> p r c", c=W)[:, :nin, :],
                        src)
                    nc.sync.dma_start(
                        xt[64:128].rearrange("p (r c) -> p r c", c=W)[:, :nin, :],
                        src)
                    gts = []
                    for q in range(NQ):
                        u = ipool.tile([128, NPIX], bf16, tag="u")
                        sq4 = ipool.tile([128, NPIX], bf16, tag="sq4")
                        an = ipool.tile([128, NPIX], bf16, tag="an")
                        m1 = ipool.tile([128, NPIX], bf16, tag="m1")
                        sa = ipool.tile([128, NPIX], bf16, tag="sa")
                        a3 = ipool.tile([128, NPIX], bf16, tag="a3")
                        r3 = ipool.tile([128, NPIX], bf16, tag="r3")
                        g = gpool.tile([128, NPIX], bf16)
                        # u = |11x - (m+2)|, m = 2q + (p>=64)
                        nc.scalar.activation(u[:, :npx], xt[:, :npx],
                                             AF.Abs, bias=bt[:, q:q + 1],
                                             scale=11.0)
                        # sq4 = (2-2u)^2 = 4(1-u)^2
                        nc.scalar.activation(sq4[:, :npx], u[:, :npx],
                                             AF.Square, bias=bt[:, NQ:NQ + 1],
                                             scale=-2.0)
                        # an = min(u-2,0) = -relu(2-u)   [DVE 4x]
                        nc.vector.tensor_scalar(an[:, :npx], u[:, :npx],
                                                2.0, 0.0, op0=alu.subtract,
                                                op1=alu.min)
                        # m1 = min(u-1,0) = -relu(1-u)   [DVE]
                        nc.vector.tensor_scalar(m1[:, :npx], u[:, :npx],
                                                1.0, 0.0, op0=alu.subtract,
                                                op1=alu.min)
                        # sa = relu(2-u)^2: GpSimd computes an*an for odd q;
                        # Scalar computes Square(2-u) for even q (equal to
                        # an*an wherever an != 0, and multiplied by an=0 else)
                        if q % 2 == 0:
                            nc.scalar.activation(sa[:, :npx], u[:, :npx],
                                                 AF.Square, bias=bt[:, NQ:NQ + 1],
                                                 scale=-1.0)
                        else:
                            nc.gpsimd.tensor_tensor(sa[:, :npx], an[:, :npx],
                                                    an[:, :npx], op=alu.mult)
                        # a3 = sa*an = -relu(2-u)^3      [DVE 2x]
                        nc.vector.tensor_tensor(a3[:, :npx], sa[:, :npx],
                                                an[:, :npx], op=alu.mult)
                        # r3 = sq4*m1 = -4relu(1-u)^3    [DVE 2x]
                        nc.vector.tensor_tensor(r3[:, :npx], sq4[:, :npx],
                                                m1[:, :npx], op=alu.mult)
                        # g = a3 - r3 = -(6*N3)          [DVE 2x]
                        nc.vector.tensor_tensor(g[:, :npx], a3[:, :npx],
                                                r3[:, :npx], op=alu.subtract)
                        gts.append(g)

                    gvs = [g[:].rearrange("p (r c) -> p r c", c=W) for g in gts]
                    n_mm = NQ * TAPS
                    for grp, nr in enumerate(groups):
                        ps = ppool.tile([64, 8, WO], f32)
                        i_mm = 0
                        for q in range(NQ):
                            for kk in range(KS):
                                for l in range(KS):
                                    r0 = 8 * grp + kk
                                    rhs = gvs[q][:, r0:r0 + nr, l:l + WO]
                                    lhsT = wts[q][:, (kk * KS + l) * COUT:
                                                  (kk * KS + l + 1) * COUT]
                                    nc.tensor.matmul(
                                        ps[:, :nr, :], lhsT, rhs,
                                        start=(i_mm == 0),
                                        stop=(i_mm == n_mm - 1),
                                    )
                                    i_mm += 1
                        ot = opool.tile([64, 8, WO], f32)
                        nc.scalar.activation(ot[:, :nr, :], ps[:, :nr, :],
                                             AF.Identity, scale=1.0)
                        nc.sync.dma_start(
                            out_d[b, :, y0 + 8 * grp:y0 + 8 * grp + nr, :],
                            ot[:, :nr, :])

    nc.compile()
    return nc


def _maybe_install_profile_shim():
    """Allow trace=True/BASS_TRACE under axon even though this image lacks
    antenv.axon_hooks; degrade silently if anything is missing."""
    import types

    if "antenv.axon_hooks" in sys.modules:
        return
    try:
        from trn_agent_boot.trn_boot import _ntff_profile_via_ctypes

        hook = _ntff_profile_via_ctypes("/opt/axon/libaxon_pjrt.so")
        if hook is None:
            return
        mod = types.ModuleType("antenv.axon_hooks")
        mod.get_axon_ntff_profile_hook = lambda: hook
        mod.set_axon_ntff_profile_hook = lambda h: None
        sys.modules["antenv.axon_hooks"] = mod
        from concourse import bass_utils

        bass_utils.upload_artifacts = lambda tmpdir: f"local:{tmpdir}"
    except Exception:
        pass


_LAST_RESULTS = None


def kernel(x: np.ndarray, coeff: np.ndarray) -> np.ndarray:
    global _LAST_RESULTS
    from concourse import bass_utils

    _maybe_install_profile_shim()

    x = np.ascontiguousarray(np.asarray(x), dtype=np.float32)
    coeff = np.asarray(coeff)
    assert x.shape == (B_FULL, CIN, H, W), x.shape

    w_host = _fold_coeff(coeff)
    btbl = np.zeros((128, NQ + 1), dtype=np.float32)
    for p in range(128):
        for q in range(NQ):
            m = 2 * q + (1 if p >= 64 else 0)
            btbl[p, q] = -float(m + 2)
    btbl[:, NQ] = 2.0

    nc = _build_bass()

    in_maps = []
    for i in range(N_CORES):
        in_maps.append({
            "x": np.ascontiguousarray(x[i * B_SHARD:(i + 1) * B_SHARD]),
            "w": w_host,
            "btbl": btbl,
        })

    res = bass_utils.run_bass_kernel_spmd(
        nc, in_maps, core_ids=list(range(N_CORES)),
        trace=bool(os.environ.get("KAN_TRACE")),
    )
    _LAST_RESULTS = res

    out = np.concatenate([res.results[i]["out"] for i in range(N_CORES)], axis=0)
    return out.astype(np.float32, copy=False)


# revision 10
# speedup vs baseline: 2.6473x; 1.0460x over previous
"""Trainium2 Bass kernel for the ConvolutionalKAN problem.

Math: the KAN conv
    out[b,o,y,x] = sum_{j,kk,l,m} phi_m(11*inp[b,j,y+kk,x+l]) * coeff[o,j,kk,l,m]
with phi_m the degree-3 B-spline basis on uniform knots linspace(0,1,12).
Uniform knots -> phi_m(t) = N3(t-m), branch-free (u = |t-(m+2)|):
    6*N3 = relu(2-u)^3 - 4*relu(1-u)^3
This kernel evaluates the NEGATED basis with clamp/square tricks so every
vector op is a cheap bf16 DVE pass (2x/4x perf modes):
    u    = |11x + b|          (Scalar, per-partition bias, bf16 out)
    sq4  = (2-2u)^2           (Scalar Square, = 4(1-u)^2)
    an   = min(u-2, 0)        (DVE tensor_scalar imm, 4x mode; = -relu(2-u))
    m1   = min(u-1, 0)        (DVE 4x; = -relu(1-u))
    sa   = an*an              (GpSimd mult; = relu(2-u)^2)
    a3   = sa*an              (DVE 2x; = -relu(2-u)^3)
    r3   = sq4*m1             (DVE 2x; = -4*relu(1-u)^3)
    g    = a3 - r3            (DVE 2x; = -(6*N3))
The sign is folded into the host-prepared weights (w = -coeff/6), so the
matmul chains (4 contraction tiles x 9 taps, bf16, N = rows*62 per PSUM
bank) are unchanged in count but all-bf16: TensorE streams 1 col/cycle at
2.4 GHz and becomes the bottleneck (~117us), with Vector ~90us, Scalar
~90us, GpSimd ~80us hidden behind it.

Sharding: data-parallel over batch, 2 images per core on 8 cores.
"""

import os
import sys

import numpy as np

for _p in ("/root/.axon_site/_ro/trn_rl_repo", "/opt/trn_rl_repo"):
    if os.path.isdir(_p) and _p not in sys.path:
        sys.path.append(_p)

B_FULL = 16
N_CORES = 8
B_SHARD = B_FULL // N_CORES
CIN = 64
COUT = 64
H = 64
W = 64
KS = 3
NB = 8
HO = H - KS + 1  # 62
WO = W - KS + 1  # 62
NQ = (CIN * NB) // 128  # 4 contraction tiles of 128
TAPS = KS * KS

# output row strips: (y0, n_input_rows, per-group output rows)
STRIPS = [(0, 18, (8, 8)), (16, 18, (8, 8)), (32, 18, (8, 8)), (48, 16, (8, 6))]


def _fold_coeff(coeff: np.ndarray):
    """coeff [COUT, CIN, KS, KS, NB] -> W_host [512, 576] f32 (negated).

    The on-chip basis is g = -(6*phi), so weights fold to -coeff/6 in
    layout W_host[m*64 + j, (kk*3+l)*64 + o].
    """
    w = (-coeff.astype(np.float64).transpose(4, 1, 2, 3, 0) / 6.0).reshape(
        NB * CIN, TAPS * COUT)
    return np.ascontiguousarray(w, dtype=np.float32)


def _build_bass():
    import concourse.bacc as bacc
    import concourse.mybir as mybir
    import concourse.tile as tile

    f32 = mybir.dt.float32
    bf16 = mybir.dt.bfloat16
    AF = mybir.ActivationFunctionType
    alu = mybir.AluOpType

    nc = bacc.Bacc("TRN2", target_bir_lowering=False, debug=False,
                   num_devices=N_CORES)
    x_d = nc.dram_tensor("x", [B_SHARD, CIN, H, W], f32, kind="ExternalInput").ap()
    w_d = nc.dram_tensor("w", [NB * CIN, TAPS * COUT], f32, kind="ExternalInput").ap()
    b_d = nc.dram_tensor("btbl", [128, NQ + 1], f32, kind="ExternalInput").ap()
    out_d = nc.dram_tensor("out", [B_SHARD, COUT, HO, WO], f32,
                           kind="ExternalOutput").ap()

    with tile.TileContext(nc) as tc:
        from contextlib import ExitStack

        with ExitStack() as ctx:
            wpool = ctx.enter_context(tc.tile_pool(name="w", bufs=NQ))
            cpool = ctx.enter_context(tc.tile_pool(name="const", bufs=1))
            xpool = ctx.enter_context(tc.tile_pool(name="x", bufs=3))
            gpool = ctx.enter_context(tc.tile_pool(name="g", bufs=3 * NQ))
            ipool = ctx.enter_context(tc.tile_pool(name="i", bufs=3))
            opool = ctx.enter_context(tc.tile_pool(name="o", bufs=4))
            ppool = ctx.enter_context(
                tc.tile_pool(name="ps", bufs=4, space="PSUM"))

            bt = cpool.tile([128, NQ + 1], f32)
            nc.sync.dma_start(bt[:], b_d[:])
            wts = []
            for q in range(NQ):
                wt = wpool.tile([128, TAPS * COUT], f32, tag="wstage")
                nc.sync.dma_start(wt[:], w_d[q * 128:(q + 1) * 128, :])
                wr = wpool.tile([128, TAPS * COUT], bf16, tag="wr")
                nc.vector.tensor_copy(wr[:], wt[:])
                wts.append(wr)

            NPIX = 18 * W
            for b in range(B_SHARD):
                for (y0, nin, groups) in STRIPS:
                    npx = nin * W
                    xt = xpool.tile([128, NPIX], f32)
                    xv = xt[:].rearrange("p (r c) -> p r c", c=W)
                    for r0c in range(0, nin, 6):
                        r1c = min(r0c + 6, nin)
                        srcc = x_d[b, :, y0 + r0c:y0 + r1c, :]
                        nc.sync.dma_start(xv[0:64, r0c:r1c, :], srcc)
                        nc.sync.dma_start(xv[64:128, r0c:r1c, :], srcc)
                    gts = []
                    for q in range(NQ):
                        u = ipool.tile([128, NPIX], bf16, tag="u")
                        sq4 = ipool.tile([128, NPIX], bf16, tag="sq4")
                        an = ipool.tile([128, NPIX], bf16, tag="an")
                        m1 = ipool.tile([128, NPIX], bf16, tag="m1")
                        sa = ipool.tile([128, NPIX], bf16, tag="sa")
                        a3 = ipool.tile([128, NPIX], bf16, tag="a3")
                        r3 = ipool.tile([128, NPIX], bf16, tag="r3")
                        g = gpool.tile([128, NPIX], bf16)
                        # u = |11x - (m+2)|, m = 2q + (p>=64)
                        nc.scalar.activation(u[:, :npx], xt[:, :npx],
                                             AF.Abs, bias=bt[:, q:q + 1],
                                             scale=11.0)
                        # sq4 = (2-2u)^2 = 4(1-u)^2
                        nc.scalar.activation(sq4[:, :npx], u[:, :npx],
                                             AF.Square, bias=bt[:, NQ:NQ + 1],
                                             scale=-2.0)
                        # an = min(u-2,0) = -relu(2-u)   [DVE 4x]
                        nc.vector.tensor_scalar(an[:, :npx], u[:, :npx],
                                                2.0, 0.0, op0=alu.subtract,
                                                op1=alu.min)
                        # m1 = min(u-1,0) = -relu(1-u)   [DVE]
                        nc.vector.tensor_scalar(m1[:, :npx], u[:, :npx],
                                                1.0, 0.0, op0=alu.subtract,
                                                op1=alu.min)
                        # sa = (2-u)^2 (= relu(2-u)^2 wherever an != 0)
                        nc.scalar.activation(sa[:, :npx], u[:, :npx],
                                             AF.Square, bias=bt[:, NQ:NQ + 1],
                                             scale=-1.0)
                        # a3 = sa*an = -relu(2-u)^3      [DVE 2x]
                        nc.vector.tensor_tensor(a3[:, :npx], sa[:, :npx],
                                                an[:, :npx], op=alu.mult)
                        # r3 = sq4*m1 = -4relu(1-u)^3    [DVE 2x]
                        nc.vector.tensor_tensor(r3[:, :npx], sq4[:, :npx],
                                                m1[:, :npx], op=alu.mult)
                        # g = a3 - r3 = -(6*N3)          [DVE 2x]
                        nc.vector.tensor_tensor(g[:, :npx], a3[:, :npx],
                                                r3[:, :npx], op=alu.subtract)
                        gts.append(g)

                    gvs = [g[:].rearrange("p (r c) -> p r c", c=W) for g in gts]
                    n_mm = NQ * TAPS
                    for grp, nr in enumerate(groups):
                        ps = ppool.tile([64, 8, WO], f32)
                        i_mm = 0
                        for q in range(NQ):
                            for kk in range(KS):
                                for l in range(KS):
                                    r0 = 8 * grp + kk
                                    rhs = gvs[q][:, r0:r0 + nr, l:l + WO]
                                    lhsT = wts[q][:, (kk * KS + l) * COUT:
                                                  (kk * KS + l + 1) * COUT]
                                    nc.tensor.matmul(
                                        ps[:, :nr, :], lhsT, rhs,
                                        start=(i_mm == 0),
                                        stop=(i_mm == n_mm - 1),
                                    )
                                    i_mm += 1
                        ot = opool.tile([64, 8, WO], f32)
                        if grp % 2 == 0:
                            nc.scalar.activation(ot[:, :nr, :], ps[:, :nr, :],
                                                 AF.Identity, scale=1.0)
                        else:
                            nc.vector.tensor_copy(ot[:, :nr, :], ps[:, :nr, :])
                        nc.sync.dma_start(
                            out_d[b, :, y0 + 8 * grp:y0 + 8 * grp + nr, :],
                            ot[:, :nr, :])

    nc.compile()
    return nc


def _maybe_install_profile_shim():
    """Allow trace=True/BASS_TRACE under axon even though this image lacks
    antenv.axon_hooks; degrade silently if anything is missing."""
    import types

    if "antenv.axon_hooks" in sys.modules:
        return
    try:
        from trn_agent_boot.trn_boot import _ntff_profile_via_ctypes

        hook = _ntff_profile_via_ctypes("/opt/axon/libaxon_pjrt.so")
        if hook is None:
            return
        mod = types.ModuleType("antenv.axon_hooks")
        mod.get_axon_ntff_profile_hook = lambda: hook
        mod.set_axon_ntff_profile_hook = lambda h: None
        sys.modules["antenv.axon_hooks"] = mod
        from concourse import bass_utils

        bass_utils.upload_artifacts = lambda tmpdir: f"local:{tmpdir}"
    except Exception:
        pass


_LAST_RESULTS = None


def kernel(x: np.ndarray, coeff: np.ndarray) -> np.ndarray:
    global _LAST_RESULTS
    from concourse import bass_utils

    _maybe_install_profile_shim()

    x = np.ascontiguousarray(np.asarray(x), dtype=np.float32)
    coeff = np.asarray(coeff)
    assert x.shape == (B_FULL, CIN, H, W), x.shape

    w_host = _fold_coeff(coeff)
    btbl = np.zeros((128, NQ + 1), dtype=np.float32)
    for p in range(128):
        for q in range(NQ):
            m = 2 * q + (1 if p >= 64 else 0)
            btbl[p, q] = -float(m + 2)
    btbl[:, NQ] = 2.0

    nc = _build_bass()

    in_maps = []
    for i in range(N_CORES):
        in_maps.append({
            "x": np.ascontiguousarray(x[i * B_SHARD:(i + 1) * B_SHARD]),
            "w": w_host,
            "btbl": btbl,
        })

    res = bass_utils.run_bass_kernel_spmd(
        nc, in_maps, core_ids=list(range(N_CORES)),
        trace=bool(os.environ.get("KAN_TRACE")),
    )
    _LAST_RESULTS = res

    out = np.concatenate([res.results[i]["out"] for i in range(N_CORES)], axis=0)
    return out.astype(np.float32, copy=False)


# revision 11
# speedup vs baseline: 2.6774x; 1.0114x over previous
"""Trainium2 Bass kernel for the ConvolutionalKAN problem.

Math: the KAN conv
    out[b,o,y,x] = sum_{j,kk,l,m} phi_m(11*inp[b,j,y+kk,x+l]) * coeff[o,j,kk,l,m]
with phi_m the degree-3 B-spline basis on uniform knots linspace(0,1,12).
Uniform knots -> phi_m(t) = N3(t-m), branch-free (u = |t-(m+2)|):
    6*N3 = relu(2-u)^3 - 4*relu(1-u)^3
This kernel evaluates the NEGATED basis with clamp/square tricks so every
vector op is a cheap bf16 DVE pass (2x/4x perf modes):
    u    = |11x + b|          (Scalar, per-partition bias, bf16 out)
    sq4  = (2-2u)^2           (Scalar Square, = 4(1-u)^2)
    an   = min(u-2, 0)        (DVE tensor_scalar imm, 4x mode; = -relu(2-u))
    m1   = min(u-1, 0)        (DVE 4x; = -relu(1-u))
    sa   = an*an              (GpSimd mult; = relu(2-u)^2)
    a3   = sa*an              (DVE 2x; = -relu(2-u)^3)
    r3   = sq4*m1             (DVE 2x; = -4*relu(1-u)^3)
    g    = a3 - r3            (DVE 2x; = -(6*N3))
The sign is folded into the host-prepared weights (w = -coeff/6), so the
matmul chains (4 contraction tiles x 9 taps, bf16, N = rows*62 per PSUM
bank) are unchanged in count but all-bf16: TensorE streams 1 col/cycle at
2.4 GHz and becomes the bottleneck (~117us), with Vector ~90us, Scalar
~90us, GpSimd ~80us hidden behind it.

Sharding: data-parallel over batch, 2 images per core on 8 cores.
"""

import os
import sys

import numpy as np

for _p in ("/root/.axon_site/_ro/trn_rl_repo", "/opt/trn_rl_repo"):
    if os.path.isdir(_p) and _p not in sys.path:
        sys.path.append(_p)

B_FULL = 16
N_CORES = 8
B_SHARD = B_FULL // N_CORES
CIN = 64
COUT = 64
H = 64
W = 64
KS = 3
NB = 8
HO = H - KS + 1  # 62
WO = W - KS + 1  # 62
NQ = (CIN * NB) // 128  # 4 contraction tiles of 128
TAPS = KS * KS

# output row strips: (y0, n_input_rows, per-group output rows)
STRIPS = [(0, 18, (8, 8)), (16, 18, (8, 8)), (32, 18, (8, 8)), (48, 16, (8, 6))]


def _fold_coeff(coeff: np.ndarray):
    """coeff [COUT, CIN, KS, KS, NB] -> W_host [512, 576] f32 (negated).

    The on-chip basis is g = -(6*phi), so weights fold to -coeff/6 in
    layout W_host[m*64 + j, (kk*3+l)*64 + o].
    """
    w = (-coeff.astype(np.float64).transpose(4, 1, 2, 3, 0) / 6.0).reshape(
        NB * CIN, TAPS * COUT)
    return np.ascontiguousarray(w, dtype=np.float32)


def _build_bass():
    import concourse.bacc as bacc
    import concourse.mybir as mybir
    import concourse.tile as tile

    f32 = mybir.dt.float32
    bf16 = mybir.dt.bfloat16
    AF = mybir.ActivationFunctionType
    alu = mybir.AluOpType

    nc = bacc.Bacc("TRN2", target_bir_lowering=False, debug=False,
                   num_devices=N_CORES)
    x_d = nc.dram_tensor("x", [B_SHARD, CIN, H, W], f32, kind="ExternalInput").ap()
    w_d = nc.dram_tensor("w", [NB * CIN, TAPS * COUT], f32, kind="ExternalInput").ap()
    b_d = nc.dram_tensor("btbl", [128, NQ + 1], f32, kind="ExternalInput").ap()
    out_d = nc.dram_tensor("out", [B_SHARD, COUT, HO, WO], f32,
                           kind="ExternalOutput").ap()

    with tile.TileContext(nc) as tc:
        from contextlib import ExitStack

        with ExitStack() as ctx:
            wpool = ctx.enter_context(tc.tile_pool(name="w", bufs=NQ))
            cpool = ctx.enter_context(tc.tile_pool(name="const", bufs=1))
            xpool = ctx.enter_context(tc.tile_pool(name="x", bufs=3))
            gpool = ctx.enter_context(tc.tile_pool(name="g", bufs=3 * NQ))
            ipool = ctx.enter_context(tc.tile_pool(name="i", bufs=3))
            opool = ctx.enter_context(tc.tile_pool(name="o", bufs=4))
            ppool = ctx.enter_context(
                tc.tile_pool(name="ps", bufs=4, space="PSUM"))

            # prefetch image 0 / strip 0 input before anything else so the
            # scalar engine can start the basis pipeline ASAP
            xt0 = xpool.tile([128, 18 * W], f32)
            xv0 = xt0[:].rearrange("p (r c) -> p r c", c=W)
            nin0 = STRIPS[0][1]
            for r0c in range(0, nin0, 6):
                r1c = min(r0c + 6, nin0)
                srcc = x_d[0, :, r0c:r1c, :]
                nc.sync.dma_start(xv0[0:64, r0c:r1c, :], srcc)
                nc.sync.dma_start(xv0[64:128, r0c:r1c, :], srcc)

            bt = cpool.tile([128, NQ + 1], f32)
            nc.sync.dma_start(bt[:], b_d[:])
            wts = []
            for q in range(NQ):
                wt = wpool.tile([128, TAPS * COUT], f32, tag="wstage")
                nc.sync.dma_start(wt[:], w_d[q * 128:(q + 1) * 128, :])
                wr = wpool.tile([128, TAPS * COUT], bf16, tag="wr")
                nc.vector.tensor_copy(wr[:], wt[:])
                wts.append(wr)

            NPIX = 18 * W
            for b in range(B_SHARD):
                for (y0, nin, groups) in STRIPS:
                    npx = nin * W
                    if b == 0 and y0 == 0:
                        xt = xt0
                    else:
                        xt = xpool.tile([128, NPIX], f32)
                        xv = xt[:].rearrange("p (r c) -> p r c", c=W)
                        for r0c in range(0, nin, 6):
                            r1c = min(r0c + 6, nin)
                            srcc = x_d[b, :, y0 + r0c:y0 + r1c, :]
                            nc.sync.dma_start(xv[0:64, r0c:r1c, :], srcc)
                            nc.sync.dma_start(xv[64:128, r0c:r1c, :], srcc)
                    gts = []
                    for q in range(NQ):
                        u = ipool.tile([128, NPIX], bf16, tag="u")
                        sq4 = ipool.tile([128, NPIX], bf16, tag="sq4")
                        an = ipool.tile([128, NPIX], bf16, tag="an")
                        m1 = ipool.tile([128, NPIX], bf16, tag="m1")
                        sa = ipool.tile([128, NPIX], bf16, tag="sa")
                        a3 = ipool.tile([128, NPIX], bf16, tag="a3")
                        r3 = ipool.tile([128, NPIX], bf16, tag="r3")
                        g = gpool.tile([128, NPIX], bf16)
                        # u = |11x - (m+2)|, m = 2q + (p>=64)
                        nc.scalar.activation(u[:, :npx], xt[:, :npx],
                                             AF.Abs, bias=bt[:, q:q + 1],
                                             scale=11.0)
                        # sq4 = (2-2u)^2 = 4(1-u)^2
                        nc.scalar.activation(sq4[:, :npx], u[:, :npx],
                                             AF.Square, bias=bt[:, NQ:NQ + 1],
                                             scale=-2.0)
                        # an = min(u-2,0) = -relu(2-u)   [DVE 4x]
                        nc.vector.tensor_scalar(an[:, :npx], u[:, :npx],
                                                2.0, 0.0, op0=alu.subtract,
                                                op1=alu.min)
                        # m1 = min(u-1,0) = -relu(1-u)   [DVE]
                        nc.vector.tensor_scalar(m1[:, :npx], u[:, :npx],
                                                1.0, 0.0, op0=alu.subtract,
                                                op1=alu.min)
                        # sa = (2-u)^2 (= relu(2-u)^2 wherever an != 0)
                        nc.scalar.activation(sa[:, :npx], u[:, :npx],
                                             AF.Square, bias=bt[:, NQ:NQ + 1],
                                             scale=-1.0)
                        # a3 = sa*an = -relu(2-u)^3      [DVE 2x]
                        nc.vector.tensor_tensor(a3[:, :npx], sa[:, :npx],
                                                an[:, :npx], op=alu.mult)
                        # r3 = sq4*m1 = -4relu(1-u)^3    [DVE 2x]
                        nc.vector.tensor_tensor(r3[:, :npx], sq4[:, :npx],
                                                m1[:, :npx], op=alu.mult)
                        # g = a3 - r3 = -(6*N3)          [DVE 2x]
                        nc.vector.tensor_tensor(g[:, :npx], a3[:, :npx],
                                                r3[:, :npx], op=alu.subtract)
                        gts.append(g)

                    gvs = [g[:].rearrange("p (r c) -> p r c", c=W) for g in gts]
                    n_mm = NQ * TAPS
                    for grp, nr in enumerate(groups):
                        ps = ppool.tile([64, 8, WO], f32)
                        i_mm = 0
                        for q in range(NQ):
                            for kk in range(KS):
                                for l in range(KS):
                                    r0 = 8 * grp + kk
                                    rhs = gvs[q][:, r0:r0 + nr, l:l + WO]
                                    lhsT = wts[q][:, (kk * KS + l) * COUT:
                                                  (kk * KS + l + 1) * COUT]
                                    nc.tensor.matmul(
                                        ps[:, :nr, :], lhsT, rhs,
                                        start=(i_mm == 0),
                                        stop=(i_mm == n_mm - 1),
                                    )
                                    i_mm += 1
                        ot = opool.tile([64, 8, WO], f32)
                        if grp % 2 == 0:
                            nc.scalar.activation(ot[:, :nr, :], ps[:, :nr, :],
                                                 AF.Identity, scale=1.0)
                        else:
                            nc.vector.tensor_copy(ot[:, :nr, :], ps[:, :nr, :])
                        nc.sync.dma_start(
                            out_d[b, :, y0 + 8 * grp:y0 + 8 * grp + nr, :],
                            ot[:, :nr, :])

    nc.compile()
    return nc


def _maybe_install_profile_shim():
    """Allow trace=True/BASS_TRACE under axon even though this image lacks
    antenv.axon_hooks; degrade silently if anything is missing."""
    import types

    if "antenv.axon_hooks" in sys.modules:
        return
    try:
        from trn_agent_boot.trn_boot import _ntff_profile_via_ctypes

        hook = _ntff_profile_via_ctypes("/opt/axon/libaxon_pjrt.so")
        if hook is None:
            return
        mod = types.ModuleType("antenv.axon_hooks")
        mod.get_axon_ntff_profile_hook = lambda: hook
        mod.set_axon_ntff_profile_hook = lambda h: None
        sys.modules["antenv.axon_hooks"] = mod
        from concourse import bass_utils

        bass_utils.upload_artifacts = lambda tmpdir: f"local:{tmpdir}"
    except Exception:
        pass


_LAST_RESULTS = None


def kernel(x: np.ndarray, coeff: np.ndarray) -> np.ndarray:
    global _LAST_RESULTS
    from concourse import bass_utils

    _maybe_install_profile_shim()

    x = np.ascontiguousarray(np.asarray(x), dtype=np.float32)
    coeff = np.asarray(coeff)
    assert x.shape == (B_FULL, CIN, H, W), x.shape

    w_host = _fold_coeff(coeff)
    btbl = np.zeros((128, NQ + 1), dtype=np.float32)
    for p in range(128):
        for q in range(NQ):
            m = 2 * q + (1 if p >= 64 else 0)
            btbl[p, q] = -float(m + 2)
    btbl[:, NQ] = 2.0

    nc = _build_bass()

    in_maps = []
    for i in range(N_CORES):
        in_maps.append({
            "x": np.ascontiguousarray(x[i * B_SHARD:(i + 1) * B_SHARD]),
            "w": w_host,
            "btbl": btbl,
        })

    res = bass_utils.run_bass_kernel_spmd(
        nc, in_maps, core_ids=list(range(N_CORES)),
        trace=bool(os.environ.get("KAN_TRACE")),
    )
    _LAST_RESULTS = res

    out = np.concatenate([res.results[i]["out"] for i in range(N_CORES)], axis=0)
    return out.astype(np.float32, copy=False)
